# revision 7
# baseline (speedup 1.0000x reference)
"""Trainium2 Bass kernel for nn_EquivariantBiLinear.

Math (per batch row b):
    Y[k, b] = sum_nu W_g[mu, nu] * x[b, bid_g[nu*r+rho]]   (k = off_g + mu*r + rho)
    out[b, o] = 0.1 * sum_i Y[W_invperm[o*256+i], b] * x[b, i]

Sharding: 4-way over batch x 2-way over k-space (8 cores; host adds the
two k-partials per batch slice). Per core: 256 chunks of 128 k-rows x
512 batch cols. Per chunk: group GEMM (fp16, N=512) -> PSUM, DVE mult
by the host-gathered xg = x[b, i_k]/4 -> z (fp16), then a one-hot fp8
scatter matmul accumulates z into the persistent output PSUM bank(s).

Key trick vs the naive layout: the scatter previously needed 2 matmuls
per chunk (its 128 target o-rows span both 128-row PSUM banks). Since
the GEMM's mu-tiling within each (group, rho)-stream is free, the host
sorts each stream's k's by target o-half so almost every chunk is
*pure* (single-bank scatter: 1 matmul). Boundary/parity chunks stay
"mixed" (2 matmuls). All weights are streamed per-chunk in the sorted
order (pure layout transform on the host). The two k-cores must share
one SPMD program, so per (stream, class) chunk counts are made even by
demoting a few pure chunks to mixed; both cores then run an identical
slot sequence with different streamed data. Scatter matmuls drop from
512 to ~308 per core.

The output o-half-0 bank finishes before the pure-1 phase, so its
epilogue (scale + transpose + stage) overlaps the remaining chunks.
"""

import sys

if "/opt/trn_rl_repo" not in sys.path:
    sys.path.insert(0, "/opt/trn_rl_repo")

from contextlib import ExitStack

import numpy as np

import concourse.bacc as bacc
import concourse.mybir as mybir
import concourse.tile as tile
from concourse.bass_utils import run_bass_kernel_spmd
from concourse.masks import make_identity

GROUPS = [(512, 1, 16384), (256, 4, 4096), (128, 16, 1024), (64, 64, 256)]
OFFS = [0, 16384, 32768, 49152]
X = 256
B = 2048
NCORES = 8
BS = 512  # batch rows per core
NSLOT = 256  # chunks per core
KT = [4, 2, 1, 1]  # K-subtile matmuls per chunk by group

F32 = mybir.dt.float32
FP16 = mybir.dt.float16
FP8 = mybir.dt.float8e4

# group g start slot (DMA warm-up: xrep_g must be resident first)
GSTART = [16, 0, 10, 26]


def _streams():
    """List of (g, stream_id, k-array in canonical order)."""
    out = []
    for gi, (n, r, m) in enumerate(GROUPS):
        off = OFFS[gi]
        if gi < 3:
            for rho in range(r):
                out.append((gi, rho, off + np.arange(m) * r + rho))
        else:
            for q in range(32):
                out.append(
                    (
                        gi,
                        q,
                        np.concatenate(
                            [
                                off + np.arange(m) * r + 2 * q,
                                off + np.arange(m) * r + 2 * q + 1,
                            ]
                        ),
                    )
                )
    return out


def _plan(perm):
    """Chunk the k-space into o-half-sorted 128-tiles and build the
    shared slot schedule. Returns (slots, chunk_of[2], meta)."""
    H = perm // (X * X // 2)

    # per (g, stream, cls) chunk lists, globally
    chunks = {}  # (g, s, cls) -> list of klist arrays
    for gi, s, ks in _streams():
        hh = H[ks]
        order = np.argsort(hh, kind="stable")
        ks = ks[order]
        n0 = int((hh == 0).sum())
        a0, c = divmod(n0, 128)
        mx = 1 if c else 0
        a1 = (len(ks) - n0) // 128
        # parity demotions: all of (a0, mx, a1) must be even
        guard = 0
        while (a0 % 2) or (mx % 2) or (a1 % 2):
            if a0 % 2:
                a0 -= 1
                mx += 1
            elif a1 % 2:
                a1 -= 1
                mx += 1
            guard += 1
            assert guard < 8
        tiles = [ks[t * 128 : (t + 1) * 128] for t in range(len(ks) // 128)]
        # first a0 are pure0, last a1 are pure1, middle mx are mixed
        chunks[(gi, s, 0)] = tiles[:a0]
        chunks[(gi, s, 1)] = tiles[a0 : a0 + mx]
        chunks[(gi, s, 2)] = tiles[a0 + mx :]
        assert len(chunks[(gi, s, 2)]) == a1

    # split each class list between the 2 k-cores (even counts)
    chunk_of = [{}, {}]
    for key, lst in chunks.items():
        assert len(lst) % 2 == 0, (key, len(lst))
        chunk_of[0][key] = lst[0::2]
        chunk_of[1][key] = lst[1::2]

    # shared slot items per phase: phase A = cls {0,1}, phase B = cls 2
    per_group = {ph: {g: [] for g in range(4)} for ph in range(2)}
    for gi, s, _ks in _streams():
        for cls in (0, 1, 2):
            nn = len(chunk_of[0][(gi, s, cls)])
            ph = 0 if cls < 2 else 1
            per_group[ph][gi].extend([(gi, s, cls)] * nn)

    # interleave groups proportionally (greedy), respecting GSTART
    slots = []
    for ph in range(2):
        lists = per_group[ph]
        tot = {g: max(1, len(lists[g])) for g in range(4)}
        idx = {g: 0 for g in range(4)}
        n_ph = sum(len(lists[g]) for g in range(4))
        for _ in range(n_ph):
            best, bestv = None, -1.0
            for g in range(4):
                if idx[g] >= len(lists[g]):
                    continue
                if ph == 0 and len(slots) < GSTART[g]:
                    continue
                v = (len(lists[g]) - idx[g]) / tot[g]
                if v > bestv:
                    best, bestv = g, v
            if best is None:  # only GSTART-blocked groups left
                for g in range(4):
                    if idx[g] < len(lists[g]):
                        best = g
                        break
            slots.append(lists[best][idx[best]])
            idx[best] += 1
    assert len(slots) == NSLOT

    # per-slot metadata (shared across cores)
    meta = []
    i0 = i1 = i23 = 0
    oh_c = 0
    for si, (gi, s, cls) in enumerate(slots):
        w = 128 if cls != 1 else 256
        if (oh_c % 1024) + w > 1024:
            oh_c = (oh_c // 1024 + 1) * 1024
        if gi == 0:
            wt = ("wt0", i0 * 512)
            i0 += 1
        elif gi == 1:
            wt = ("wt1", i1 * 256)
            i1 += 1
        else:
            wt = ("wt23", i23 * 128)
            i23 += 1
        banks = [0] if cls == 0 else ([1] if cls == 2 else [0, 1])
        meta.append(
            dict(g=gi, s=s, cls=cls, wt=wt, oh=oh_c, oh_w=w, banks=banks)
        )
        oh_c += w
    ohw = (oh_c + 1023) // 1024 * 1024

    # start/stop per bank
    first = {0: None, 1: None}
    last = {0: None, 1: None}
    for si, m in enumerate(meta):
        for bk in m["banks"]:
            if first[bk] is None:
                first[bk] = si
            last[bk] = si
    for si, m in enumerate(meta):
        m["start"] = {bk: si == first[bk] for bk in m["banks"]}
        m["stop"] = {bk: si == last[bk] for bk in m["banks"]}
    ep0_after = last[0]

    counts = (i0, i1, i23)
    return slots, chunk_of, meta, ohw, ep0_after, counts


def _host_prep(W0, W1, W2, W3, bid0, bid1, bid2, bid3, W_invperm):
    """Pure layout transforms of weights/indices (no arithmetic on data)."""
    Ws = [np.asarray(W) for W in (W0, W1, W2, W3)]
    bids = [np.asarray(b).astype(np.int64) for b in (bid0, bid1, bid2, bid3)]
    ivp = np.asarray(W_invperm).astype(np.int64)
    perm = np.empty(X * X, np.int64)
    perm[ivp] = np.arange(X * X)

    slots, chunk_of, meta, ohw, ep0_after, counts = _plan(perm)

    # weight matrices transposed to (n, m) fp16
    wt = [
        np.ascontiguousarray(W.reshape(m, n).T.astype(np.float16))
        for (n, r, m), W in zip(GROUPS, Ws)
    ]

    # per-core streamed planes + xg row-index lists
    import ml_dtypes

    e4m3 = ml_dtypes.float8_e4m3
    pr = np.arange(128)
    cores = []
    for kc in range(2):
        qidx = {key: 0 for key in chunk_of[kc]}
        wt0p = np.zeros((128, counts[0] * 512), np.float16)
        wt1p = np.zeros((128, counts[1] * 256), np.float16)
        wt23p = np.zeros((128, counts[2] * 128), np.float16)
        ohp = np.zeros((128, ohw), e4m3)
        iks = np.empty(NSLOT * 128, np.int64)
        for si, m in enumerate(meta):
            gi, s, cls = slots[si]
            ks = chunk_of[kc][(gi, s, cls)][qidx[(gi, s, cls)]]
            qidx[(gi, s, cls)] += 1
            off = OFFS[gi]
            n, r, _m = GROUPS[gi]
            # lhsT columns
            kind, woff = m["wt"]
            if gi == 3:
                mu = (ks - off) // r
                rho = (ks - off) % r
                sub = rho - 2 * s  # 0 or 1 within the pair
                assert np.all((sub == 0) | (sub == 1))
                blk = np.zeros((128, 128), np.float16)
                rows = sub[None, :] * 64 + np.arange(64)[:, None]  # (64, 128)
                blk[rows, np.arange(128)[None, :]] = wt[3][:, mu]
                wt23p[:, woff : woff + 128] = blk
            else:
                mu = (ks - off) // r
                cols = wt[gi][:, mu]  # (n, 128)
                if gi == 0:
                    for kcc in range(4):
                        wt0p[:, woff + kcc * 128 : woff + (kcc + 1) * 128] = (
                            cols[kcc * 128 : (kcc + 1) * 128, :]
                        )
                elif gi == 1:
                    for kcc in range(2):
                        wt1p[:, woff + kcc * 128 : woff + (kcc + 1) * 128] = (
                            cols[kcc * 128 : (kcc + 1) * 128, :]
                        )
                else:
                    wt23p[:, woff : woff + 128] = cols
            # one-hot + xg indices
            pk = perm[ks]
            iks[si * 128 : (si + 1) * 128] = pk % X
            ok = pk // X
            ooff = m["oh"]
            if cls == 1:
                bk = ok // 128
                ohp[pr, ooff + bk * 128 + ok % 128] = 1.0
            else:
                ohp[pr, ooff + ok % 128] = 1.0
        cores.append(dict(wt0=wt0p, wt1=wt1p, wt23=wt23p, oh=ohp, iks=iks))

    # x-gather column map for xrep (identical to reference layout)
    colsl = []
    b0 = bids[0]
    for kcc in range(4):
        colsl.append(b0[kcc * 128 : (kcc + 1) * 128])
    b1 = bids[1].reshape(256, 4)
    for kcc in range(2):
        for rho in range(4):
            colsl.append(b1[kcc * 128 : (kcc + 1) * 128, rho])
    b2 = bids[2].reshape(128, 16)
    for rho in range(16):
        colsl.append(b2[:, rho])
    b3 = bids[3].reshape(64, 64)
    for q in range(32):
        colsl.append(b3[pr % 64, 2 * q + pr // 64])
    xgidx = np.ascontiguousarray(np.stack(colsl, axis=1).astype(np.int64))

    plan_key = (
        tuple(slots),
        tuple((m["oh"], m["oh_w"]) for m in meta),
        ohw,
        ep0_after,
        counts,
    )
    plan = dict(
        slots=slots, meta=meta, ohw=ohw, ep0_after=ep0_after, counts=counts,
        key=hash(repr(plan_key)),
    )
    return plan, cores, xgidx


def _build_nc(plan):
    meta = plan["meta"]
    ohw = plan["ohw"]
    ep0_after = plan["ep0_after"]
    n0s, n1s, n23s = plan["counts"]

    nc = bacc.Bacc("TRN2", target_bir_lowering=False, debug=False, num_devices=NCORES)

    xrep_d = nc.dram_tensor("xrep", [128, 60 * BS], FP16, kind="ExternalInput")
    xg_d = nc.dram_tensor("xg", [128, NSLOT * BS], FP16, kind="ExternalInput")
    wt0_d = nc.dram_tensor("wt0", [128, n0s * 512], FP16, kind="ExternalInput")
    wt1_d = nc.dram_tensor("wt1", [128, n1s * 256], FP16, kind="ExternalInput")
    wt23_d = nc.dram_tensor("wt23", [128, n23s * 128], FP16, kind="ExternalInput")
    oh_d = nc.dram_tensor("oh", [128, ohw], FP8, kind="ExternalInput")
    out_d = nc.dram_tensor("out", [BS, X], F32, kind="ExternalOutput")

    # ---- precompute fetch schedule -------------------------------------
    # per-slot resources: xg(pair), wt0/wt1 per slot, wt23 + oh windows.
    PF = 3  # pairs of lookahead
    npair = NSLOT // 2
    fetch = [[] for _ in range(npair)]  # pair -> list of (kind, arg)

    for p in range(npair):
        fetch[p].append(("xg", p))
    i23 = 0
    w23_seen = set()
    oh_seen = set()
    for si, m in enumerate(meta):
        p = si // 2
        kind, woff = m["wt"]
        if kind == "wt0":
            fetch[p].append(("wt0", woff))
        elif kind == "wt1":
            fetch[p].append(("wt1", woff))
        else:
            w = woff // 512
            if w not in w23_seen:
                w23_seen.add(w)
                fetch[p].append(("wt23", w))
            i23 += 1
        w = m["oh"] // 1024
        if w not in oh_seen:
            oh_seen.add(w)
            fetch[p].append(("oh", w))
    n_w23 = (n23s * 128 + 511) // 512
    n_ohw = ohw // 1024

    with tile.TileContext(nc) as tc, ExitStack() as ctx:
        const = ctx.enter_context(tc.tile_pool(name="const", bufs=1))
        w0pool = ctx.enter_context(tc.tile_pool(name="w0pool", bufs=6))
        w1pool = ctx.enter_context(tc.tile_pool(name="w1pool", bufs=6))
        w23pool = ctx.enter_context(tc.tile_pool(name="w23pool", bufs=3))
        xgpool = ctx.enter_context(tc.tile_pool(name="xgpool", bufs=5))
        ohpool = ctx.enter_context(tc.tile_pool(name="ohpool", bufs=4))
        ypool = ctx.enter_context(tc.tile_pool(name="ypool", bufs=4))
        zpool = ctx.enter_context(tc.tile_pool(name="zpool", bufs=8))
        pgemm = ctx.enter_context(tc.tile_pool(name="pgemm", bufs=3, space="PSUM"))
        pout = ctx.enter_context(tc.tile_pool(name="pout", bufs=1, space="PSUM"))

        ident = const.tile([128, 128], F32)
        make_identity(nc, ident[:])

        # xrep resident tiles; per-kc tiles for g0/g1 so the first GEMMs
        # wait only on their own slice.
        xrep0t = [const.tile([128, BS], FP16, name=f"xr0_{i}") for i in range(4)]
        xrep1t = [const.tile([128, 4 * BS], FP16, name=f"xr1_{i}") for i in range(2)]
        xrep2t = const.tile([128, 16 * BS], FP16, name="xr2")
        xrep3t = const.tile([128, 32 * BS], FP16, name="xr3")

        # ring byte counters for balancing
        rb = {"sync": 0, "scalar": 0}

        def ring(nbytes, prefer=None):
            if prefer is None:
                prefer = "sync" if rb["sync"] <= rb["scalar"] else "scalar"
            rb[prefer] += nbytes
            return nc.sync if prefer == "sync" else nc.scalar

        state = {
            "xgq": [], "w0q": [], "w1q": [], "w23q": [], "ohq": [],
            "w23_cur": None, "w23_left": 0, "w23_off": 0,
            "pend": [], "ps": None,
        }

        def do_fetch(p):
            if p >= npair:
                return
            for kind, arg in fetch[p]:
                if kind == "xg":
                    t = xgpool.tile([128, 1024], FP16, tag="xgt", name="xgt")
                    eng = ring(262144, "sync" if arg % 8 else "scalar")
                    eng.dma_start(t[:], xg_d[:, arg * 1024 : (arg + 1) * 1024])
                    state["xgq"].append(t)
                elif kind == "wt0":
                    t = w0pool.tile([128, 512], FP16, tag="w0t", name="w0t")
                    ring(131072).dma_start(t[:], wt0_d[:, arg : arg + 512])
                    state["w0q"].append(t)
                elif kind == "wt1":
                    t = w1pool.tile([128, 256], FP16, tag="w1t", name="w1t")
                    ring(65536).dma_start(t[:], wt1_d[:, arg : arg + 256])
                    state["w1q"].append(t)
                elif kind == "wt23":
                    t = w23pool.tile([128, 512], FP16, tag="w23t", name="w23t")
                    hi = min((arg + 1) * 512, n23s * 128)
                    ring(131072).dma_start(t[:, : hi - arg * 512], wt23_d[:, arg * 512 : hi])
                    state["w23q"].append(t)
                elif kind == "oh":
                    t = ohpool.tile([128, 1024], FP8, tag="oht", name="oht")
                    ring(131072).dma_start(t[:], oh_d[:, arg * 1024 : (arg + 1) * 1024])
                    state["ohq"].append((arg, t))

        # persistent output accumulators: one PSUM BANK per o-half
        outT_ps = [
            pout.tile([128, BS], F32, tag=f"pout{h}", name=f"pout{h}") for h in range(2)
        ]
        outstage = [
            const.tile([128, 256], F32, name=f"outstage{bh}") for bh in range(4)
        ]

        def flush_pending():
            for si, ohs, zv in state["pend"]:
                m = meta[si]
                for bk in m["banks"]:
                    if m["cls"] == 1:
                        lhs = ohs[:, bk * 128 : (bk + 1) * 128]
                    else:
                        lhs = ohs[:, 0:128]
                    nc.tensor.matmul(
                        outT_ps[bk][:],
                        lhs,
                        zv,
                        start=m["start"][bk],
                        stop=m["stop"][bk],
                        skip_group_check=True,
                    )
            state["pend"].clear()

        def epilogue(hb):
            outT_sb = zpool.tile([128, BS], F32, tag="outT_sb", name="outT_sb", bufs=2)
            nc.scalar.mul(outT_sb[:], outT_ps[hb][:], 0.4)
            for bh in range(4):
                pst2 = pgemm.tile([128, 1024], F32, tag="pg", name="pst2")
                nc.tensor.transpose(
                    pst2[:, 0:128], outT_sb[:, bh * 128 : (bh + 1) * 128], ident[:]
                )
                nc.any.tensor_copy(
                    outstage[bh][:, hb * 128 : (hb + 1) * 128], pst2[:, 0:128]
                )

        # ---- startup: xrep + first fetches, interleaved by priority ----
        # g1 (slots 0..): xrep1 first; then g2 (xrep2), g0, g3.
        nc.sync.dma_start(xrep1t[0][:], xrep_d[:, 4 * BS : 8 * BS])
        nc.scalar.dma_start(xrep1t[1][:], xrep_d[:, 8 * BS : 12 * BS])
        do_fetch(0)
        do_fetch(1)
        nc.sync.dma_start(xrep2t[:, 0 : 8 * BS], xrep_d[:, 12 * BS : 20 * BS])
        nc.scalar.dma_start(xrep2t[:, 8 * BS : 16 * BS], xrep_d[:, 20 * BS : 28 * BS])
        do_fetch(2)
        for i in range(4):
            eng = nc.sync if i % 2 == 0 else nc.scalar
            eng.dma_start(xrep0t[i][:], xrep_d[:, i * BS : (i + 1) * BS])
        rb["sync"] += 8 * BS * 2 * 128 + 2 * BS * 2 * 128
        rb["scalar"] += 8 * BS * 2 * 128 + 2 * BS * 2 * 128
        for i in range(4):
            eng = nc.sync if i % 2 == 0 else nc.scalar
            eng.dma_start(
                xrep3t[:, i * 8 * BS : (i + 1) * 8 * BS],
                xrep_d[:, (28 + i * 8) * BS : (28 + (i + 1) * 8) * BS],
            )
        rb["sync"] += 16 * BS * 2 * 128
        rb["scalar"] += 16 * BS * 2 * 128

        # ---- main loop over slots ----
        for si, m in enumerate(meta):
            gi = m["g"]
            s = m["s"]
            if si % 2 == 0:
                do_fetch(si // 2 + PF)
                ps = pgemm.tile([128, 1024], F32, tag="pg", name="ps")
                state["ps"] = ps
            ps_half = state["ps"][:, (si % 2) * 512 : (si % 2 + 1) * 512]

            # GEMM
            if gi == 0:
                w = state["w0q"].pop(0)
                for kcc in range(4):
                    nc.tensor.matmul(
                        ps_half,
                        w[:, kcc * 128 : (kcc + 1) * 128],
                        xrep0t[kcc][:],
                        start=(kcc == 0),
                        stop=(kcc == 3),
                    )
            elif gi == 1:
                w = state["w1q"].pop(0)
                for kcc in range(2):
                    nc.tensor.matmul(
                        ps_half,
                        w[:, kcc * 128 : (kcc + 1) * 128],
                        xrep1t[kcc][:, s * BS : (s + 1) * BS],
                        start=(kcc == 0),
                        stop=(kcc == 1),
                    )
            else:
                if state["w23_left"] == 0:
                    state["w23_cur"] = state["w23q"].pop(0)
                    state["w23_left"] = 4
                    state["w23_off"] = 0
                w = state["w23_cur"]
                o = state["w23_off"]
                lhs = w[:, o * 128 : (o + 1) * 128]
                state["w23_left"] -= 1
                state["w23_off"] += 1
                rhs = (
                    xrep2t[:, s * BS : (s + 1) * BS]
                    if gi == 2
                    else xrep3t[:, s * BS : (s + 1) * BS]
                )
                nc.tensor.matmul(ps_half, lhs, rhs, start=True, stop=True)

            if si % 2 == 1:
                # copy pair PSUM -> SBUF fp16, mult by xg -> z
                if len(state["pend"]) >= 6:
                    flush_pending()
                yt = ypool.tile([128, 1024], FP16, tag="yt", name="yt")
                pair = si // 2
                if pair < 6 or pair % 10 < 3:
                    nc.vector.tensor_copy(yt[:], state["ps"][:])
                else:
                    nc.scalar.copy(yt[:], state["ps"][:])
                xgt = state["xgq"].pop(0)
                z16 = zpool.tile([128, 1024], FP16, tag="z16", name="z16")
                nc.vector.tensor_mul(z16[:], yt[:], xgt[:])
                for j in (si - 1, si):
                    mm = meta[j]
                    while state["ohq"] and state["ohq"][0][0] < mm["oh"] // 1024:
                        state["ohq"].pop(0)
                    assert state["ohq"] and state["ohq"][0][0] == mm["oh"] // 1024
                    oht = state["ohq"][0][1]
                    ooff = mm["oh"] % 1024
                    ohs = oht[:, ooff : ooff + mm["oh_w"]]
                    zv = z16[:, (j % 2) * 512 : (j % 2 + 1) * 512]
                    state["pend"].append((j, ohs, zv))
                if si == ep0_after or si - 1 == ep0_after:
                    flush_pending()
                    epilogue(0)

        flush_pending()
        epilogue(1)
        for bh in range(4):
            nc.sync.dma_start(out_d[bh * 128 : (bh + 1) * 128, :], outstage[bh][:])

    nc.compile()
    return nc


_NC_CACHE = None  # (key, nc)


def _make_in_maps(x, plan, cores, xgidx):
    x = np.ascontiguousarray(np.asarray(x, dtype=np.float32))
    in_maps = []
    for c in range(NCORES):
        bc, kc = divmod(c, 2)
        xsh = x[bc * BS : (bc + 1) * BS, :]
        xr = xsh[:, xgidx]  # (512 b, 128 nu, 60 t)
        xrep = np.ascontiguousarray(
            xr.transpose(1, 2, 0).reshape(128, 60 * BS).astype(np.float16)
        )
        co = cores[kc]
        A = (xsh[:, co["iks"]].T / 4.0).astype(np.float16)  # (NSLOT*128, 512)
        xg = np.ascontiguousarray(
            A.reshape(NSLOT, 128, BS).transpose(1, 0, 2).reshape(128, NSLOT * BS)
        )
        in_maps.append(
            {
                "xrep": xrep,
                "xg": xg,
                "oh": co["oh"],
                "wt0": co["wt0"],
                "wt1": co["wt1"],
                "wt23": co["wt23"],
            }
        )
    return in_maps


def kernel(x, W0, W1, W2, W3, bid0, bid1, bid2, bid3, W_invperm, **_unused):
    global _NC_CACHE
    plan, cores, xgidx = _host_prep(
        W0, W1, W2, W3, bid0, bid1, bid2, bid3, W_invperm
    )
    if _NC_CACHE is None or _NC_CACHE[0] != plan["key"]:
        _NC_CACHE = (plan["key"], _build_nc(plan))
    nc = _NC_CACHE[1]

    in_maps = _make_in_maps(x, plan, cores, xgidx)
    res = run_bass_kernel_spmd(nc, in_maps, core_ids=list(range(NCORES)))
    outs = [np.asarray(res.results[c]["out"], np.float32) for c in range(NCORES)]
    out = np.concatenate(
        [outs[2 * bc] + outs[2 * bc + 1] for bc in range(NCORES // 2)], axis=0
    )
    return out.astype(np.float32)


# revision 8
# speedup vs baseline: 1.1864x; 1.1864x over previous
"""Trainium2 Bass kernel for nn_EquivariantBiLinear.

Math (per batch row b):
    Y[k, b] = sum_nu W_g[mu, nu] * x[b, bid_g[nu*r+rho]]   (k = off_g + mu*r + rho)
    out[b, o] = 0.1 * sum_i Y[W_invperm[o*256+i], b] * x[b, i]

Sharding: 4-way over batch x 2-way over k-space (8 cores; host adds the
two k-partials per batch slice). Per core: 256 chunks of 128 k-rows x
512 batch cols. Per chunk: group GEMM (fp16, N=512) -> PSUM, DVE mult
by the host-gathered xg = x[b, i_k]/4 -> z (fp16), then a one-hot fp8
scatter matmul accumulates z into the persistent output PSUM bank(s).

Key trick vs the naive layout: the scatter previously needed 2 matmuls
per chunk (its 128 target o-rows span both 128-row PSUM banks). Since
the GEMM's mu-tiling within each (group, rho)-stream is free, the host
sorts each stream's k's by target o-half so almost every chunk is
*pure* (single-bank scatter: 1 matmul). Boundary/parity chunks stay
"mixed" (2 matmuls). All weights are streamed per-chunk in the sorted
order (pure layout transform on the host). The two k-cores must share
one SPMD program, so per (stream, class) chunk counts are made even by
demoting a few pure chunks to mixed; both cores then run an identical
slot sequence with different streamed data. Scatter matmuls drop from
512 to ~308 per core.

Scheduling: slots are ordered big(g0/g1)-small(g2/g3) alternating so
every PSUM-pair step carries enough PE work to hide the PSUM->SBUF
copy; xrep is loaded as 60 per-(group,rho) blocks posted in first-use
order so the first GEMM starts ~3us in; DMA fetches are batched into
>=1KB/partition slabs to cut sequencer post overhead. The o-half-0
output bank finishes before the pure-1 phase, so its epilogue overlaps
the remaining chunks.
"""

import sys

if "/opt/trn_rl_repo" not in sys.path:
    sys.path.insert(0, "/opt/trn_rl_repo")

from contextlib import ExitStack

import numpy as np

import concourse.bacc as bacc
import concourse.mybir as mybir
import concourse.tile as tile
from concourse.bass_utils import run_bass_kernel_spmd
from concourse.masks import make_identity

GROUPS = [(512, 1, 16384), (256, 4, 4096), (128, 16, 1024), (64, 64, 256)]
OFFS = [0, 16384, 32768, 49152]
X = 256
B = 2048
NCORES = 8
BS = 512  # batch rows per core
NSLOT = 256  # chunks per core
OHWIN = 2048  # one-hot fetch window (fp8 cols)

F32 = mybir.dt.float32
FP16 = mybir.dt.float16
FP8 = mybir.dt.float8e4

# earliest slot index per group (xrep warm-up)
GSTART = [4, 0, 1, 9]


def _streams():
    """List of (g, stream_id, k-array in canonical order)."""
    out = []
    for gi, (n, r, m) in enumerate(GROUPS):
        off = OFFS[gi]
        if gi < 3:
            for rho in range(r):
                out.append((gi, rho, off + np.arange(m) * r + rho))
        else:
            for q in range(32):
                out.append(
                    (
                        gi,
                        q,
                        np.concatenate(
                            [
                                off + np.arange(m) * r + 2 * q,
                                off + np.arange(m) * r + 2 * q + 1,
                            ]
                        ),
                    )
                )
    return out


def _order_slots(per_group):
    """Merge the 4 per-group item lists into one slot order: big (g0/g1)
    and small (g2/g3) alternate so PSUM pairs carry steady PE work;
    within each size class groups interleave proportionally; GSTART
    delays a group until its xrep blocks can be resident."""

    def mk_queue(groups):
        lists = {g: list(per_group[g]) for g in groups}
        tot = {g: max(1, len(lists[g])) for g in groups}
        idx = {g: 0 for g in groups}

        def pop(slot_i):
            best, bestv = None, -1.0
            for g in groups:
                if idx[g] >= len(lists[g]) or slot_i < GSTART[g]:
                    continue
                v = (len(lists[g]) - idx[g]) / tot[g]
                if v > bestv:
                    best, bestv = g, v
            if best is None:
                for g in groups:
                    if idx[g] < len(lists[g]):
                        best = g
                        break
            if best is None:
                return None
            it = lists[best][idx[best]]
            idx[best] += 1
            return it

        return pop

    nb = len(per_group[0]) + len(per_group[1])
    ns = len(per_group[2]) + len(per_group[3])
    big = mk_queue([1, 0])
    small = mk_queue([2, 3])
    out = []
    want_big = False
    for _ in range(nb + ns):
        want_big = not want_big
        it = (big if want_big else small)(len(out))
        if it is None:
            it = (small if want_big else big)(len(out))
        out.append(it)
    return out


def _plan(perm):
    """Chunk the k-space into o-half-sorted 128-tiles and build the
    shared slot schedule."""
    H = perm // (X * X // 2)

    chunks = {}  # (g, s, cls) -> list of klist arrays
    for gi, s, ks in _streams():
        hh = H[ks]
        order = np.argsort(hh, kind="stable")
        ks = ks[order]
        n0 = int((hh == 0).sum())
        a0, c = divmod(n0, 128)
        mx = 1 if c else 0
        a1 = (len(ks) - n0) // 128
        guard = 0
        while (a0 % 2) or (mx % 2) or (a1 % 2):
            if a0 % 2:
                a0 -= 1
                mx += 1
            elif a1 % 2:
                a1 -= 1
                mx += 1
            guard += 1
            assert guard < 8
        tiles = [ks[t * 128 : (t + 1) * 128] for t in range(len(ks) // 128)]
        chunks[(gi, s, 0)] = tiles[:a0]
        chunks[(gi, s, 1)] = tiles[a0 : a0 + mx]
        chunks[(gi, s, 2)] = tiles[a0 + mx :]

    chunk_of = [{}, {}]
    for key, lst in chunks.items():
        assert len(lst) % 2 == 0, (key, len(lst))
        chunk_of[0][key] = lst[0::2]
        chunk_of[1][key] = lst[1::2]

    per_group = {ph: {g: [] for g in range(4)} for ph in range(2)}
    for gi, s, _ks in _streams():
        for cls in (0, 1, 2):
            nn = len(chunk_of[0][(gi, s, cls)])
            ph = 0 if cls < 2 else 1
            per_group[ph][gi].extend([(gi, s, cls)] * nn)

    slots = _order_slots(per_group[0]) + _order_slots(per_group[1])
    assert len(slots) == NSLOT

    meta = []
    i0 = i1 = i23 = 0
    oh_c = 0
    for si, (gi, s, cls) in enumerate(slots):
        w = 128 if cls != 1 else 256
        if (oh_c % OHWIN) + w > OHWIN:
            oh_c = (oh_c // OHWIN + 1) * OHWIN
        if gi == 0:
            wt = ("wt0", i0 * 512)
            i0 += 1
        elif gi == 1:
            wt = ("wt1", i1 * 256)
            i1 += 1
        else:
            wt = ("wt23", i23 * 128)
            i23 += 1
        banks = [0] if cls == 0 else ([1] if cls == 2 else [0, 1])
        meta.append(dict(g=gi, s=s, cls=cls, wt=wt, oh=oh_c, oh_w=w, banks=banks))
        oh_c += w
    ohw = (oh_c + OHWIN - 1) // OHWIN * OHWIN

    first = {0: None, 1: None}
    last = {0: None, 1: None}
    for si, m in enumerate(meta):
        for bk in m["banks"]:
            if first[bk] is None:
                first[bk] = si
            last[bk] = si
    for si, m in enumerate(meta):
        m["start"] = {bk: si == first[bk] for bk in m["banks"]}
        m["stop"] = {bk: si == last[bk] for bk in m["banks"]}
    ep0_after = last[0]

    counts = (i0, i1, i23)
    return slots, chunk_of, meta, ohw, ep0_after, counts


def _host_prep(W0, W1, W2, W3, bid0, bid1, bid2, bid3, W_invperm):
    """Pure layout transforms of weights/indices (no arithmetic on data)."""
    import ml_dtypes

    Ws = [np.asarray(W) for W in (W0, W1, W2, W3)]
    bids = [np.asarray(b).astype(np.int64) for b in (bid0, bid1, bid2, bid3)]
    ivp = np.asarray(W_invperm).astype(np.int64)
    perm = np.empty(X * X, np.int64)
    perm[ivp] = np.arange(X * X)

    slots, chunk_of, meta, ohw, ep0_after, counts = _plan(perm)

    wt = [
        np.ascontiguousarray(W.reshape(m, n).T.astype(np.float16))
        for (n, r, m), W in zip(GROUPS, Ws)
    ]

    e4m3 = ml_dtypes.float8_e4m3
    pr = np.arange(128)
    cores = []
    for kc in range(2):
        qidx = {key: 0 for key in chunk_of[kc]}
        wt0p = np.zeros((128, counts[0] * 512), np.float16)
        wt1p = np.zeros((128, counts[1] * 256), np.float16)
        wt23p = np.zeros((128, counts[2] * 128), np.float16)
        ohp = np.zeros((128, ohw), e4m3)
        iks = np.empty(NSLOT * 128, np.int64)
        for si, m in enumerate(meta):
            gi, s, cls = slots[si]
            ks = chunk_of[kc][(gi, s, cls)][qidx[(gi, s, cls)]]
            qidx[(gi, s, cls)] += 1
            off = OFFS[gi]
            n, r, _m = GROUPS[gi]
            kind, woff = m["wt"]
            if gi == 3:
                mu = (ks - off) // r
                rho = (ks - off) % r
                sub = rho - 2 * s  # 0 or 1 within the pair
                assert np.all((sub == 0) | (sub == 1))
                blk = np.zeros((128, 128), np.float16)
                rows = sub[None, :] * 64 + np.arange(64)[:, None]  # (64, 128)
                blk[rows, np.arange(128)[None, :]] = wt[3][:, mu]
                wt23p[:, woff : woff + 128] = blk
            else:
                mu = (ks - off) // r
                cols = wt[gi][:, mu]  # (n, 128)
                if gi == 0:
                    for kcc in range(4):
                        wt0p[:, woff + kcc * 128 : woff + (kcc + 1) * 128] = cols[
                            kcc * 128 : (kcc + 1) * 128, :
                        ]
                elif gi == 1:
                    for kcc in range(2):
                        wt1p[:, woff + kcc * 128 : woff + (kcc + 1) * 128] = cols[
                            kcc * 128 : (kcc + 1) * 128, :
                        ]
                else:
                    wt23p[:, woff : woff + 128] = cols
            pk = perm[ks]
            iks[si * 128 : (si + 1) * 128] = pk % X
            ok = pk // X
            ooff = m["oh"]
            if cls == 1:
                bk = ok // 128
                ohp[pr, ooff + bk * 128 + ok % 128] = 1.0
            else:
                ohp[pr, ooff + ok % 128] = 1.0
        cores.append(dict(wt0=wt0p, wt1=wt1p, wt23=wt23p, oh=ohp, iks=iks))

    # x-gather column map for xrep (identical to reference layout)
    colsl = []
    b0 = bids[0]
    for kcc in range(4):
        colsl.append(b0[kcc * 128 : (kcc + 1) * 128])
    b1 = bids[1].reshape(256, 4)
    for kcc in range(2):
        for rho in range(4):
            colsl.append(b1[kcc * 128 : (kcc + 1) * 128, rho])
    b2 = bids[2].reshape(128, 16)
    for rho in range(16):
        colsl.append(b2[:, rho])
    b3 = bids[3].reshape(64, 64)
    for q in range(32):
        colsl.append(b3[pr % 64, 2 * q + pr // 64])
    xgidx = np.ascontiguousarray(np.stack(colsl, axis=1).astype(np.int64))

    plan_key = (
        tuple(slots),
        tuple((m["oh"], m["oh_w"]) for m in meta),
        ohw,
        ep0_after,
        counts,
    )
    plan = dict(
        slots=slots, meta=meta, ohw=ohw, ep0_after=ep0_after, counts=counts,
        key=hash(repr(plan_key)),
    )
    return plan, cores, xgidx


def _xrep_block(gi, s, kcc=0):
    """xrep plane block index for (group, stream[, kc])."""
    if gi == 0:
        return kcc
    if gi == 1:
        return 4 + kcc * 4 + s
    if gi == 2:
        return 12 + s
    return 28 + s


def _build_nc(plan):
    meta = plan["meta"]
    ohw = plan["ohw"]
    ep0_after = plan["ep0_after"]
    n0s, n1s, n23s = plan["counts"]

    nc = bacc.Bacc("TRN2", target_bir_lowering=False, debug=False, num_devices=NCORES)

    xrep_d = nc.dram_tensor("xrep", [128, 60 * BS], FP16, kind="ExternalInput")
    xg_d = nc.dram_tensor("xg", [128, NSLOT * BS], FP16, kind="ExternalInput")
    wt0_d = nc.dram_tensor("wt0", [128, n0s * 512], FP16, kind="ExternalInput")
    wt1_d = nc.dram_tensor("wt1", [128, n1s * 256], FP16, kind="ExternalInput")
    wt23_d = nc.dram_tensor("wt23", [128, n23s * 128], FP16, kind="ExternalInput")
    oh_d = nc.dram_tensor("oh", [128, ohw], FP8, kind="ExternalInput")
    out_d = nc.dram_tensor("out", [BS, X], F32, kind="ExternalOutput")

    # ---- fetch schedule: batched slabs, attached to the pair that is
    # PF pairs ahead of first use (negative -> preamble) ----
    PF = 4
    npair = NSLOT // 2
    fetch = [[] for _ in range(npair)]
    pre = []  # preamble fetches, in priority order

    def sched(first_use_pair, op):
        p = first_use_pair - PF
        if p < 0:
            pre.append((first_use_pair, op))
        else:
            fetch[p].append(op)

    # xrep blocks, at first use
    xrep_first = {}
    for si, m in enumerate(meta):
        gi, s = m["g"], m["s"]
        blocks = (
            [_xrep_block(0, 0, kcc) for kcc in range(4)]
            if gi == 0
            else (
                [_xrep_block(1, s, kcc) for kcc in range(2)]
                if gi == 1
                else [_xrep_block(gi, s)]
            )
        )
        for blk in blocks:
            if blk not in xrep_first:
                xrep_first[blk] = si // 2
    for blk, fu in sorted(xrep_first.items(), key=lambda kv: kv[1]):
        sched(fu, ("xrep", blk))

    # xg slabs of 2 pairs
    for slab in range((npair + 1) // 2):
        sched(slab * 2, ("xg", slab))
    # wt windows: wt0 [2 g0-slots], wt1 [2 g1-slots], wt23 [8 slots]
    seen = set()
    for si, m in enumerate(meta):
        kind, woff = m["wt"]
        if kind == "wt0":
            w = woff // 1024
        elif kind == "wt1":
            w = woff // 512
        else:
            w = woff // 1024
        if (kind, w) not in seen:
            seen.add((kind, w))
            sched(si // 2, (kind, w))
    # oh windows
    seen_oh = set()
    for si, m in enumerate(meta):
        w = m["oh"] // OHWIN
        if w not in seen_oh:
            seen_oh.add(w)
            sched(si // 2, ("oh", w))
    pre.sort(key=lambda kv: kv[0])

    with tile.TileContext(nc) as tc, ExitStack() as ctx:
        const = ctx.enter_context(tc.tile_pool(name="const", bufs=1))
        w0pool = ctx.enter_context(tc.tile_pool(name="w0pool", bufs=4))
        w1pool = ctx.enter_context(tc.tile_pool(name="w1pool", bufs=4))
        w23pool = ctx.enter_context(tc.tile_pool(name="w23pool", bufs=3))
        xgpool = ctx.enter_context(tc.tile_pool(name="xgpool", bufs=4))
        ohpool = ctx.enter_context(tc.tile_pool(name="ohpool", bufs=3))
        ypool = ctx.enter_context(tc.tile_pool(name="ypool", bufs=4))
        zpool = ctx.enter_context(tc.tile_pool(name="zpool", bufs=8))
        pgemm = ctx.enter_context(tc.tile_pool(name="pgemm", bufs=3, space="PSUM"))
        pout = ctx.enter_context(tc.tile_pool(name="pout", bufs=1, space="PSUM"))

        ident = const.tile([128, 128], F32)
        make_identity(nc, ident[:])

        xrept = [const.tile([128, BS], FP16, name=f"xr{b}") for b in range(60)]

        rb = {"sync": 0, "scalar": 0}

        def ring(nbytes):
            prefer = "sync" if rb["sync"] <= rb["scalar"] else "scalar"
            rb[prefer] += nbytes
            return nc.sync if prefer == "sync" else nc.scalar

        state = {
            "xgq": [], "w0q": [], "w1q": [], "w23q": [], "ohq": [],
            "xg_cur": None, "xg_off": 0,
            "w0_cur": None, "w0_off": 2, "w1_cur": None, "w1_off": 2,
            "w23_cur": None, "w23_off": 8,
            "pend": [], "ps": None,
        }

        def do_op(op):
            kind, arg = op
            if kind == "xrep":
                ring(BS * 256).dma_start(
                    xrept[arg][:], xrep_d[:, arg * BS : (arg + 1) * BS]
                )
            elif kind == "xg":
                t = xgpool.tile([128, 2048], FP16, tag="xgt", name="xgt")
                lo = arg * 2048
                ring(524288).dma_start(t[:], xg_d[:, lo : lo + 2048])
                state["xgq"].append(t)
            elif kind == "wt0":
                t = w0pool.tile([128, 1024], FP16, tag="w0t", name="w0t")
                lo = arg * 1024
                hi = min(lo + 1024, n0s * 512)
                ring((hi - lo) * 256).dma_start(t[:, : hi - lo], wt0_d[:, lo:hi])
                state["w0q"].append(t)
            elif kind == "wt1":
                t = w1pool.tile([128, 512], FP16, tag="w1t", name="w1t")
                lo = arg * 512
                hi = min(lo + 512, n1s * 256)
                ring((hi - lo) * 256).dma_start(t[:, : hi - lo], wt1_d[:, lo:hi])
                state["w1q"].append(t)
            elif kind == "wt23":
                t = w23pool.tile([128, 1024], FP16, tag="w23t", name="w23t")
                lo = arg * 1024
                hi = min(lo + 1024, n23s * 128)
                ring((hi - lo) * 256).dma_start(t[:, : hi - lo], wt23_d[:, lo:hi])
                state["w23q"].append(t)
            elif kind == "oh":
                t = ohpool.tile([128, OHWIN], FP8, tag="oht", name="oht")
                lo = arg * OHWIN
                ring(OHWIN * 128).dma_start(t[:], oh_d[:, lo : lo + OHWIN])
                state["ohq"].append((arg, t))

        outT_ps = [
            pout.tile([128, BS], F32, tag=f"pout{h}", name=f"pout{h}") for h in range(2)
        ]
        outstage = [
            const.tile([128, 256], F32, name=f"outstage{bh}") for bh in range(4)
        ]

        def flush_pending():
            for si, ohs, zv in state["pend"]:
                m = meta[si]
                for bk in m["banks"]:
                    if m["cls"] == 1:
                        lhs = ohs[:, bk * 128 : (bk + 1) * 128]
                    else:
                        lhs = ohs[:, 0:128]
                    nc.tensor.matmul(
                        outT_ps[bk][:],
                        lhs,
                        zv,
                        start=m["start"][bk],
                        stop=m["stop"][bk],
                        skip_group_check=True,
                    )
            state["pend"].clear()

        def epilogue(hb):
            outT_sb = zpool.tile([128, BS], F32, tag="outT_sb", name="outT_sb", bufs=2)
            nc.scalar.mul(outT_sb[:], outT_ps[hb][:], 0.4)
            for bh in range(4):
                pst2 = pgemm.tile([128, 1024], F32, tag="pg", name="pst2")
                nc.tensor.transpose(
                    pst2[:, 0:128], outT_sb[:, bh * 128 : (bh + 1) * 128], ident[:]
                )
                nc.any.tensor_copy(
                    outstage[bh][:, hb * 128 : (hb + 1) * 128], pst2[:, 0:128]
                )

        # ---- preamble fetches (startup-critical, ring-alternating) ----
        for _, op in pre:
            do_op(op)

        # ---- main loop over slots ----
        for si, m in enumerate(meta):
            gi = m["g"]
            s = m["s"]
            if si % 2 == 0:
                for op in fetch[si // 2]:
                    do_op(op)
                ps = pgemm.tile([128, 1024], F32, tag="pg", name="ps")
                state["ps"] = ps
            ps_half = state["ps"][:, (si % 2) * 512 : (si % 2 + 1) * 512]

            # GEMM
            if gi == 0:
                if state["w0_off"] == 2:
                    state["w0_cur"] = state["w0q"].pop(0)
                    state["w0_off"] = 0
                w = state["w0_cur"]
                o = state["w0_off"] * 512
                state["w0_off"] += 1
                for kcc in range(4):
                    nc.tensor.matmul(
                        ps_half,
                        w[:, o + kcc * 128 : o + (kcc + 1) * 128],
                        xrept[_xrep_block(0, 0, kcc)][:],
                        start=(kcc == 0),
                        stop=(kcc == 3),
                    )
            elif gi == 1:
                if state["w1_off"] == 2:
                    state["w1_cur"] = state["w1q"].pop(0)
                    state["w1_off"] = 0
                w = state["w1_cur"]
                o = state["w1_off"] * 256
                state["w1_off"] += 1
                for kcc in range(2):
                    nc.tensor.matmul(
                        ps_half,
                        w[:, o + kcc * 128 : o + (kcc + 1) * 128],
                        xrept[_xrep_block(1, s, kcc)][:],
                        start=(kcc == 0),
                        stop=(kcc == 1),
                    )
            else:
                if state["w23_off"] == 8:
                    state["w23_cur"] = state["w23q"].pop(0)
                    state["w23_off"] = 0
                w = state["w23_cur"]
                o = state["w23_off"] * 128
                state["w23_off"] += 1
                nc.tensor.matmul(
                    ps_half,
                    w[:, o : o + 128],
                    xrept[_xrep_block(gi, s)][:],
                    start=True,
                    stop=True,
                )

            if si % 2 == 1:
                if len(state["pend"]) >= 6:
                    flush_pending()
                yt = ypool.tile([128, 1024], FP16, tag="yt", name="yt")
                pair = si // 2
                if pair < 6 or pair % 4 == 3:
                    nc.vector.tensor_copy(yt[:], state["ps"][:])
                else:
                    nc.scalar.copy(yt[:], state["ps"][:])
                if state["xg_off"] == 2 or state["xg_cur"] is None:
                    state["xg_cur"] = state["xgq"].pop(0)
                    state["xg_off"] = 0
                xgt = state["xg_cur"]
                xo = state["xg_off"] * 1024
                state["xg_off"] += 1
                z16 = zpool.tile([128, 1024], FP16, tag="z16", name="z16")
                nc.vector.tensor_mul(z16[:], yt[:], xgt[:, xo : xo + 1024])
                for j in (si - 1, si):
                    mm = meta[j]
                    while state["ohq"] and state["ohq"][0][0] < mm["oh"] // OHWIN:
                        state["ohq"].pop(0)
                    assert state["ohq"] and state["ohq"][0][0] == mm["oh"] // OHWIN
                    oht = state["ohq"][0][1]
                    ooff = mm["oh"] % OHWIN
                    ohs = oht[:, ooff : ooff + mm["oh_w"]]
                    zv = z16[:, (j % 2) * 512 : (j % 2 + 1) * 512]
                    state["pend"].append((j, ohs, zv))
                if si == ep0_after or si - 1 == ep0_after:
                    flush_pending()
                    epilogue(0)

        flush_pending()
        epilogue(1)
        for bh in range(4):
            nc.sync.dma_start(out_d[bh * 128 : (bh + 1) * 128, :], outstage[bh][:])

    nc.compile()
    return nc


_NC_CACHE = None  # (key, nc)


def _make_in_maps(x, plan, cores, xgidx):
    x = np.ascontiguousarray(np.asarray(x, dtype=np.float32))
    in_maps = []
    for c in range(NCORES):
        bc, kc = divmod(c, 2)
        xsh = x[bc * BS : (bc + 1) * BS, :]
        xr = xsh[:, xgidx]  # (512 b, 128 nu, 60 t)
        xrep = np.ascontiguousarray(
            xr.transpose(1, 2, 0).reshape(128, 60 * BS).astype(np.float16)
        )
        co = cores[kc]
        A = (xsh[:, co["iks"]].T / 4.0).astype(np.float16)  # (NSLOT*128, 512)
        xg = np.ascontiguousarray(
            A.reshape(NSLOT, 128, BS).transpose(1, 0, 2).reshape(128, NSLOT * BS)
        )
        in_maps.append(
            {
                "xrep": xrep,
                "xg": xg,
                "oh": co["oh"],
                "wt0": co["wt0"],
                "wt1": co["wt1"],
                "wt23": co["wt23"],
            }
        )
    return in_maps


def kernel(x, W0, W1, W2, W3, bid0, bid1, bid2, bid3, W_invperm, **_unused):
    global _NC_CACHE
    plan, cores, xgidx = _host_prep(
        W0, W1, W2, W3, bid0, bid1, bid2, bid3, W_invperm
    )
    if _NC_CACHE is None or _NC_CACHE[0] != plan["key"]:
        _NC_CACHE = (plan["key"], _build_nc(plan))
    nc = _NC_CACHE[1]

    in_maps = _make_in_maps(x, plan, cores, xgidx)
    res = run_bass_kernel_spmd(nc, in_maps, core_ids=list(range(NCORES)))
    outs = [np.asarray(res.results[c]["out"], np.float32) for c in range(NCORES)]
    out = np.concatenate(
        [outs[2 * bc] + outs[2 * bc + 1] for bc in range(NCORES // 2)], axis=0
    )
    return out.astype(np.float32)
